# revision 27
# baseline (speedup 1.0000x reference)
"""Causal self-attention on 8 TRN2 NeuronCores.

Sharding: batch (2) x head-groups (4 heads each) -> 8 cores. Each core
computes the qkv projection for its 4 heads, causal attention over
lower-triangular 128-blocks, and a partial o-projection. Host transposes the
per-head attention maps (device emits them j-major), sums o partials across
head groups, and adds the output bias.

Scores are computed only in transposed layout [j, i]: softmax row-sums come
from a ones-column appended to V (matmul row 64), 1/sum is broadcast across
partitions with gpsimd, and normalization is a tensor-tensor multiply. This
keeps the PE stream pure fp32r matmuls (no transpose-mode ops that confuse
the HAM clock gate) and each attention element passes through ACT exactly
once (the exp) plus one DVE multiply.

Self-contained: hardcodes B=2, T=2048, C=1024, H=16, D=64.
"""

import numpy as np
from contextlib import ExitStack

import concourse.bass as bass
import concourse.tile as tile
import concourse.mybir as mybir
from concourse import bacc
import concourse.bass_utils as bass_utils

F32 = mybir.dt.float32
F32R = mybir.dt.float32r
AF = mybir.ActivationFunctionType
ALU = mybir.AluOpType

P = 128
T = 2048
C = 1024
D = 64
HL = 4          # local heads per core
NQKV = 3 * HL * D   # 768
TB = T // P     # 16 row blocks
CB = C // P     # 8 contraction chunks for qkv
MB = NQKV // P  # 6 qkv output row-blocks
TCH = T // 512  # 4 column chunks of 512
NEG = -1e30


def build_nc():
    nc = bacc.Bacc("TRN2", target_bir_lowering=False, debug=False)

    x = nc.dram_tensor("x", [T, C], F32, kind="ExternalInput").ap()
    w3 = nc.dram_tensor("w3", [C, NQKV], F32, kind="ExternalInput").ap()
    b3 = nc.dram_tensor("b3", [NQKV], F32, kind="ExternalInput").ap()
    wo = nc.dram_tensor("wo", [HL * D, C], F32, kind="ExternalInput").ap()
    ident_in = nc.dram_tensor("ident", [P, P], F32, kind="ExternalInput").ap()
    id2_in = nc.dram_tensor("id2", [P, D], F32, kind="ExternalInput").ap()
    maskT_in = nc.dram_tensor("maskT", [P, P], F32, kind="ExternalInput").ap()
    ones4_in = nc.dram_tensor("ones4", [P, HL], F32, kind="ExternalInput").ap()

    # attention, transposed per head: attn4T[h, j, i] = attn_w[h, i, j]
    attn4T = nc.dram_tensor("attn4T", [HL, T, T], F32, kind="ExternalOutput").ap()
    o_part = nc.dram_tensor("o_part", [T, C], F32, kind="ExternalOutput").ap()

    with ExitStack() as ctx:
        tc = ctx.enter_context(tile.TileContext(nc))
        const = ctx.enter_context(tc.tile_pool(name="const", bufs=1))
        persist = ctx.enter_context(tc.tile_pool(name="persist", bufs=1))
        srow = ctx.enter_context(tc.tile_pool(name="srow", bufs=2))
        pool_mm = ctx.enter_context(tc.tile_pool(name="pmm", bufs=2, space="PSUM"))

        # ---- constants ----
        ident = const.tile([P, P], F32)
        nc.sync.dma_start(ident[:], ident_in)
        id2 = const.tile([P, D], F32)
        nc.sync.dma_start(id2[:], id2_in)
        id2_r = const.tile([P, D], F32R)
        nc.vector.tensor_copy(id2_r[:], id2[:])
        ident_r = const.tile([P, P], F32R)
        nc.vector.tensor_copy(ident_r[:], ident[:])
        maskT = const.tile([P, P], F32)
        nc.sync.dma_start(maskT[:], maskT_in)
        ones4 = const.tile([P, HL], F32)
        nc.sync.dma_start(ones4[:], ones4_in)
        ones4_r = const.tile([P, HL], F32R)
        nc.vector.tensor_copy(ones4_r[:], ones4[:])

        b_sb = const.tile([P, MB], F32)
        nc.sync.dma_start(b_sb[:], b3.rearrange("(o p) -> p o", p=P))

        # rounded weights (fp32r matmul operands must be compute-produced)
        qT_pair = persist.tile([P, 2, T], F32R)   # [q0q1 | q2q3]
        # per-head kT padded to K=128 with zeros in the other head's rows, so
        # score matmuls light up the full PE array (keeps the HAM clock warm)
        kT_pad = persist.tile([P, HL, T], F32R)
        # v natural + ones col; padded so lhsT can over-read to M=128 (junk
        # output rows 65..127 of psav are never consumed)
        v1 = persist.tile([P, TB * HL * (D + 1) + P, ], F32R)

        ab_ctx = ExitStack()
        pool_w = ab_ctx.enter_context(tc.tile_pool(name="pw", bufs=1))
        pool_vt = ab_ctx.enter_context(tc.tile_pool(name="pvt", bufs=1))
        w_r = pool_w.tile([P, CB, NQKV], F32R)
        vT_tmp = pool_vt.tile([P, 2, T], F32R)

        with tc.tile_pool(name="zp", bufs=1) as zpool:
            zc = zpool.tile([D, 512], F32)
            nc.gpsimd.memset(zc[:], 0.0)
            for h in range(HL):
                zoff = 0 if h % 2 else D
                for tch in range(TCH):
                    nc.any.tensor_copy(
                        kT_pad[zoff:zoff + D, h, tch * 512:(tch + 1) * 512],
                        zc[:],
                    )

        # ---- phase AB: per 512-col chunk: transpose x, project to qkvT ----

        with (
            tc.tile_pool(name="xld", bufs=2) as pool_x,
            tc.tile_pool(name="xtc", bufs=2) as pool_xt,
            tc.tile_pool(name="ptr", bufs=3, space="PSUM") as pool_tr,
            tc.tile_pool(name="wtmp", bufs=2) as wtmp,
        ):
            for tch in range(TCH):
                xT_chunk = pool_xt.tile([P, CB, 512], F32R)
                for tbl in range(4):
                    tb = tch * 4 + tbl
                    x_tb = pool_x.tile([P, C], F32)
                    nc.sync.dma_start(x_tb[:], x[tb * P:(tb + 1) * P, :])
                    for cbp in range(2):   # pack 4 transposes per psum bank
                        pst = pool_tr.tile([P, 512], F32, tag="tr")
                        for k in range(4):
                            cb = cbp * 4 + k
                            nc.tensor.transpose(
                                pst[:, k * P:(k + 1) * P],
                                x_tb[:, cb * P:(cb + 1) * P],
                                ident[:],
                            )
                        for k in range(4):
                            cb = cbp * 4 + k
                            nc.any.tensor_copy(
                                xT_chunk[:, cb, tbl * P:(tbl + 1) * P],
                                pst[:, k * P:(k + 1) * P],
                            )
                if tch == 0:
                    # weight loads queue behind the first x tiles on DMA
                    for cb in range(CB):
                        t_ = wtmp.tile([P, NQKV], F32, tag="wld")
                        nc.sync.dma_start(t_[:], w3[cb * P:(cb + 1) * P, :])
                        nc.any.tensor_copy(w_r[:, cb, :], t_[:])
                for mb in range(MB):
                    psq = pool_mm.tile([P, 1024], F32, tag="mm",
                                       name=f"psq_{tch}_{mb}")[:, :512]
                    for cb in range(CB):
                        nc.tensor.matmul(
                            psq[:],
                            w_r[:, cb, mb * P:(mb + 1) * P],
                            xT_chunk[:, cb, :],
                            start=(cb == 0),
                            stop=(cb == CB - 1),
                        )
                    ts512 = slice(tch * 512, (tch + 1) * 512)
                    if mb < 2:
                        nc.scalar.activation(
                            qT_pair[:, mb, ts512], psq[:],
                            AF.Identity, bias=b_sb[:, mb:mb + 1], scale=1.0,
                        )
                    elif mb < 4:
                        for half in range(2):
                            h = (mb - 2) * 2 + half
                            nc.scalar.activation(
                                kT_pad[half * D:(half + 1) * D, h, ts512],
                                psq[half * D:(half + 1) * D, :],
                                AF.Identity, bias=b_sb[half * D:(half + 1) * D,
                                                       mb:mb + 1],
                                scale=1.0,
                            )
                    else:
                        nc.scalar.activation(
                            vT_tmp[:, mb - 4, ts512], psq[:],
                            AF.Identity, bias=b_sb[:, mb:mb + 1], scale=1.0,
                        )
                # v rows of this 512-col chunk -> natural layout
                # (head pairs interleaved so base-0/base-64 transposes overlap)
                for hp2 in range(2):
                    psv = {}
                    for h in (2 * hp2, 2 * hp2 + 1):
                        psv[h] = pool_tr.tile(
                            [P, 512], F32R, tag="tr", name=f"psv{h}"
                        )
                    for k in range(4):
                        jb = tch * 4 + k
                        for h in (2 * hp2, 2 * hp2 + 1):
                            base = D * (h % 2)
                            vT_h = vT_tmp[base:base + D, h // 2, :]
                            nc.tensor.transpose(
                                psv[h][:, k * D:(k + 1) * D],
                                vT_h[:, jb * P:(jb + 1) * P],
                                id2_r[base:base + D, :],
                                tile_position=(base, 0),
                            )
                    for k in range(4):
                        jb = tch * 4 + k
                        for h in (2 * hp2, 2 * hp2 + 1):
                            off = (jb * HL + h) * (D + 1)
                            nc.any.tensor_copy(
                                v1[:, off:off + D],
                                psv[h][:, k * D:(k + 1) * D],
                            )
                for k in range(4):
                    jb = tch * 4 + k
                    nc.any.tensor_copy(
                        v1[:, jb * HL * (D + 1) + D:
                              (jb * HL + HL) * (D + 1):D + 1].rearrange(
                            "p h -> p h"),
                        ones4_r[:],
                    )

        ab_ctx.close()

        wo_pair = persist.tile([P, 2, C], F32R)
        with tc.tile_pool(name="wotmp", bufs=2) as wtmp2:
            for hp in range(2):
                t_ = wtmp2.tile([P, C], F32, tag="wold")
                nc.sync.dma_start(t_[:], wo[hp * P:(hp + 1) * P, :])
                nc.any.tensor_copy(wo_pair[:, hp, :], t_[:])

        # ---- phase C: attention, software-pipelined waves ----
        # wave = (head-pair, 512-col i-chunk). While wave W's scores stream
        # through PE->ACT(exp), wave W-1's attn_v matmuls keep PE dense.
        # Raw exp tiles go to DRAM in groups of 4 j-blocks (one 3D DMA);
        # the host folds the softmax division into its transpose pass.
        # attn_v is normalized on device, so o is exact. The o-projection
        # for chunk c is emitted one wave after both head-pairs' avT for c
        # are ready, keeping PE dense to the end.
        # avT[0:64, hp, :] = even head of pair hp; [64:128, hp, :] = odd head
        avT = persist.tile([P, 2, T], F32R)
        waves = [(hp, c) for hp in range(2) for c in range(4)]

        def emit_av(Wm1, jb, psav, eTus):
            hp, c = Wm1
            njb = 4 * c + 4
            for h in (2 * hp, 2 * hp + 1):
                off = (jb * HL + h) * (D + 1)
                nc.tensor.matmul(
                    psav[h][:],
                    v1[:, off:off + P],
                    eTus[h][jb // 4][:, jb % 4, :],
                    start=(jb == 0), stop=(jb == njb - 1),
                )

        def emit_tail(Wm1, psav):
            hp, c = Wm1
            for h in (2 * hp, 2 * hp + 1):
                sums_sb = srow.tile([D + 1, 512], F32, tag="sums",
                                    name=f"sums_{h}_{c}")
                nc.vector.tensor_copy(sums_sb[D:D + 1, :], psav[h][D:D + 1, :])
                rsp = srow.tile([P, 4], F32, tag="rsp", name=f"rsp_{h}_{c}")
                nc.sync.dma_start(rsp[:], sums_sb[D:D + 1, :])
                nc.vector.reciprocal(rsp[:], rsp[:])
                row0 = srow.tile([1, 512], F32, tag="row0", name=f"row0_{h}_{c}")
                nc.sync.dma_start(row0[:], rsp[:])
                rb = srow.tile([P, 512], F32, tag="rb", name=f"rb_{h}_{c}")
                nc.gpsimd.partition_broadcast(rb[:], row0[:])
                k = h % 2
                nc.vector.tensor_tensor(
                    avT[k * D:(k + 1) * D, hp, c * 512:(c + 1) * 512],
                    psav[h][0:D, :], rb[0:D, :], ALU.mult,
                )

        def emit_o(c, pool_o):
            for tb in range(4 * c, 4 * c + 4):
                for ncol in range(2):
                    pso = pool_mm.tile([P, 1024], F32, tag="mm",
                                       name=f"pso_{tb}_{ncol}")[:, :512]
                    for hp in range(2):
                        nc.tensor.matmul(
                            pso[:],
                            avT[:, hp, tb * P:(tb + 1) * P],
                            wo_pair[:, hp, ncol * 512:(ncol + 1) * 512],
                            start=(hp == 0), stop=(hp == 1),
                        )
                    o_sb = pool_o.tile([P, 512], F32, tag="osb",
                                       name=f"osb_{tb}_{ncol}")
                    nc.vector.tensor_copy(o_sb[:], pso[:])
                    nc.sync.dma_start(
                        o_part[tb * P:(tb + 1) * P, ncol * 512:(ncol + 1) * 512],
                        o_sb[:],
                    )

        with tc.tile_pool(name="eTu", bufs=11) as pool_eT, \
             tc.tile_pool(name="osbp", bufs=3) as pool_o, \
             tc.tile_pool(name="pav", bufs=4, space="PSUM") as pool_av:
            prev = None   # (W, psav, eTus)
            pending_o = None
            wide = {}
            for W in waves:
                hp, c = W
                heads = (2 * hp, 2 * hp + 1)
                rhs_q = qT_pair[:, hp, :]
                njb = 4 * c + 4
                njb_prev = (4 * prev[0][1] + 4) if prev else 0
                if pending_o is not None:
                    emit_o(pending_o, pool_o)
                    pending_o = None
                psav = {h: pool_av.tile([P, 512], F32, tag="av",
                                        name=f"psav_{h}_{c}")
                        for h in heads}
                eTus = {h: [] for h in heads}
                for jb in range(max(njb, njb_prev)):
                    if jb < njb:
                        g = jb // 4
                        if jb % 4 == 0:
                            for h in heads:
                                eTus[h].append(pool_eT.tile(
                                    [P, 4, 512], F32R, tag="eTu",
                                    name=f"eTu_{h}_{c}_{g}"))
                        if jb % 2 == 0:
                            wide.clear()
                            for h in heads:
                                wide[h] = pool_mm.tile(
                                    [P, 1024], F32, tag="mm",
                                    name=f"pss_{h}_{c}_{jb}")
                        for h in heads:
                            nc.tensor.matmul(
                                wide[h][:, (jb % 2) * 512:(jb % 2) * 512 + 512],
                                kT_pad[:, h, jb * P:(jb + 1) * P],
                                rhs_q[:, c * 512:(c + 1) * 512],
                                start=True, stop=True,
                            )
                        p = jb - 4 * c
                        for h in heads:
                            off = (jb % 2) * 512
                            ps = wide[h][:, off:off + 512]
                            if p >= 0:
                                if p > 0:
                                    nc.vector.tensor_scalar_add(
                                        ps[:, :p * P], ps[:, :p * P], NEG
                                    )
                                nc.vector.tensor_tensor(
                                    ps[:, p * P:(p + 1) * P],
                                    ps[:, p * P:(p + 1) * P],
                                    maskT[:], ALU.add,
                                )
                        if jb % 2 == 1:
                            for h in heads:
                                nc.scalar.activation(
                                    eTus[h][g][:, (jb % 4) - 1:(jb % 4) + 1, :],
                                    wide[h][:], AF.Exp, scale=0.125
                                )
                        if jb % 4 == 3:
                            for h in heads:
                                nc.sync.dma_start(
                                    attn4T[h, g * 512:(g + 1) * 512,
                                           c * 512:(c + 1) * 512].rearrange(
                                        "(jb p) n -> p jb n", p=P),
                                    eTus[h][g][:].bitcast(F32),
                                )
                    if prev is not None and jb < njb_prev:
                        emit_av(prev[0], jb, prev[1], prev[2])
                if prev is not None:
                    emit_tail(prev[0], prev[1])
                    if prev[0][0] == 1:
                        pending_o = prev[0][1]
                prev = (W, psav, eTus)
            # drain last wave
            if pending_o is not None:
                emit_o(pending_o, pool_o)
            for jb in range(4 * prev[0][1] + 4):
                emit_av(prev[0], jb, prev[1], prev[2])
            emit_tail(prev[0], prev[1])
            emit_o(prev[0][1], pool_o)

    nc.compile()
    return nc


_NC_CACHE = []


def _get_nc():
    if not _NC_CACHE:
        _NC_CACHE.append(build_nc())
    return _NC_CACHE[0]


def _host_consts():
    ident = np.eye(P, dtype=np.float32)
    id2 = np.concatenate([np.eye(D, dtype=np.float32)] * 2, axis=0)
    maskT = np.tril(np.full((P, P), NEG, dtype=np.float32), -1)
    ones4 = np.ones((P, HL), dtype=np.float32)
    return ident, id2, maskT, ones4


def kernel(x, w_qkv, b_qkv, w_o, b_o, _trace=False, _trace_kwargs=None):
    x = np.ascontiguousarray(np.asarray(x, dtype=np.float32))
    w_qkv = np.asarray(w_qkv, dtype=np.float32)
    b_qkv = np.asarray(b_qkv, dtype=np.float32)
    w_o = np.asarray(w_o, dtype=np.float32)
    b_o = np.asarray(b_o, dtype=np.float32)

    H = 16
    ident, id2, maskT, ones4 = _host_consts()
    in_maps = []
    for core in range(8):
        b = core // 4
        hg = (core % 4) * HL
        cols = np.r_[hg * D:(hg + HL) * D]
        w3 = np.concatenate(
            [w_qkv[:, cols], w_qkv[:, C + cols], w_qkv[:, 2 * C + cols]], axis=1
        )
        b3 = np.concatenate(
            [b_qkv[cols], b_qkv[C + cols], b_qkv[2 * C + cols]]
        )
        in_maps.append({
            "x": np.ascontiguousarray(x[b]),
            "w3": np.ascontiguousarray(w3),
            "b3": np.ascontiguousarray(b3),
            "wo": np.ascontiguousarray(w_o[hg * D:(hg + HL) * D, :]),
            "ident": ident,
            "id2": id2,
            "maskT": maskT,
            "ones4": ones4,
        })

    nc = _get_nc()
    kw = {}
    if _trace:
        kw = dict(trace=True, **(_trace_kwargs or {}))
    res = bass_utils.run_bass_kernel_spmd(
        nc, in_maps, core_ids=list(range(8)), **kw
    )

    attn_w = np.empty((2, H, T, T), dtype=np.float32)
    o = np.zeros((2, T, C), dtype=np.float32)
    for core in range(8):
        b = core // 4
        hg = (core % 4) * HL
        r = res.results[core]
        for h in range(HL):
            aw = np.ascontiguousarray(r["attn4T"][h].T)   # [i, j] raw exp
            s = aw.sum(axis=1, keepdims=True)
            np.divide(aw, s, out=aw)
            attn_w[b, hg + h] = aw
        o[b] += r["o_part"]
    o += b_o

    if _trace:
        return (o, attn_w), res
    return o, attn_w


# revision 28
# speedup vs baseline: 1.0703x; 1.0703x over previous
"""Causal self-attention on 8 TRN2 NeuronCores.

Sharding: batch (2) x head-groups (4 heads each) -> 8 cores. Each core
computes the qkv projection for its 4 heads, causal attention over
lower-triangular 128-blocks, and a partial o-projection. Host transposes the
per-head attention maps (device emits them j-major), sums o partials across
head groups, and adds the output bias.

Scores are computed only in transposed layout [j, i]: softmax row-sums come
from a ones-column appended to V (matmul row 64), 1/sum is broadcast across
partitions with gpsimd, and normalization is a tensor-tensor multiply. This
keeps the PE stream pure fp32r matmuls (no transpose-mode ops that confuse
the HAM clock gate) and each attention element passes through ACT exactly
once (the exp) plus one DVE multiply.

Self-contained: hardcodes B=2, T=2048, C=1024, H=16, D=64.
"""

import numpy as np
from contextlib import ExitStack

import concourse.bass as bass
import concourse.tile as tile
import concourse.mybir as mybir
from concourse import bacc
import concourse.bass_utils as bass_utils

F32 = mybir.dt.float32
F32R = mybir.dt.float32r
AF = mybir.ActivationFunctionType
ALU = mybir.AluOpType

P = 128
T = 2048
C = 1024
D = 64
HL = 4          # local heads per core
NQKV = 3 * HL * D   # 768
TB = T // P     # 16 row blocks
CB = C // P     # 8 contraction chunks for qkv
MB = NQKV // P  # 6 qkv output row-blocks
TCH = T // 512  # 4 column chunks of 512
NEG = -1e30


def build_nc():
    nc = bacc.Bacc("TRN2", target_bir_lowering=False, debug=False)

    x = nc.dram_tensor("x", [T, C], F32, kind="ExternalInput").ap()
    w3 = nc.dram_tensor("w3", [C, NQKV], F32, kind="ExternalInput").ap()
    b3 = nc.dram_tensor("b3", [NQKV], F32, kind="ExternalInput").ap()
    wo = nc.dram_tensor("wo", [HL * D, C], F32, kind="ExternalInput").ap()
    ident_in = nc.dram_tensor("ident", [P, P], F32, kind="ExternalInput").ap()
    id2_in = nc.dram_tensor("id2", [P, D], F32, kind="ExternalInput").ap()
    maskT_in = nc.dram_tensor("maskT", [P, P], F32, kind="ExternalInput").ap()
    ones4_in = nc.dram_tensor("ones4", [P, HL], F32, kind="ExternalInput").ap()

    # attention, transposed per head: attn4T[h, j, i] = attn_w[h, i, j]
    attn4T = nc.dram_tensor("attn4T", [HL, T, T], F32, kind="ExternalOutput").ap()
    o_part = nc.dram_tensor("o_part", [T, C], F32, kind="ExternalOutput").ap()

    with ExitStack() as ctx:
        tc = ctx.enter_context(tile.TileContext(nc))
        const = ctx.enter_context(tc.tile_pool(name="const", bufs=1))
        persist = ctx.enter_context(tc.tile_pool(name="persist", bufs=1))
        srow = ctx.enter_context(tc.tile_pool(name="srow", bufs=2))
        pool_mm = ctx.enter_context(tc.tile_pool(name="pmm", bufs=4, space="PSUM"))

        # ---- constants ----
        ident = const.tile([P, P], F32)
        nc.sync.dma_start(ident[:], ident_in)
        id2 = const.tile([P, D], F32)
        nc.sync.dma_start(id2[:], id2_in)
        id2_r = const.tile([P, D], F32R)
        nc.vector.tensor_copy(id2_r[:], id2[:])
        ident_r = const.tile([P, P], F32R)
        nc.vector.tensor_copy(ident_r[:], ident[:])
        maskT = const.tile([P, P], F32)
        nc.sync.dma_start(maskT[:], maskT_in)
        ones4 = const.tile([P, HL], F32)
        nc.sync.dma_start(ones4[:], ones4_in)
        ones4_r = const.tile([P, HL], F32R)
        nc.vector.tensor_copy(ones4_r[:], ones4[:])

        b_sb = const.tile([P, MB], F32)
        nc.sync.dma_start(b_sb[:], b3.rearrange("(o p) -> p o", p=P))

        # rounded weights (fp32r matmul operands must be compute-produced)
        qT_pair = persist.tile([P, 2, T], F32R)   # [q0q1 | q2q3]
        # per-head kT padded to K=128 with zeros in the other head's rows, so
        # score matmuls light up the full PE array (keeps the HAM clock warm)
        kT_pad = persist.tile([P, HL, T], F32R)
        # v natural + ones col; padded so lhsT can over-read to M=128 (junk
        # output rows 65..127 of psav are never consumed)
        v1 = persist.tile([P, TB * HL * (D + 1) + P, ], F32R)

        ab_ctx = ExitStack()
        pool_w = ab_ctx.enter_context(tc.tile_pool(name="pw", bufs=1))
        pool_vt = ab_ctx.enter_context(tc.tile_pool(name="pvt", bufs=1))
        w_r = pool_w.tile([P, CB, NQKV], F32R)
        vT_tmp = pool_vt.tile([P, 2, T], F32R)

        with tc.tile_pool(name="zp", bufs=1) as zpool:
            zc = zpool.tile([D, 512], F32)
            nc.gpsimd.memset(zc[:], 0.0)
            for h in range(HL):
                zoff = 0 if h % 2 else D
                for tch in range(TCH):
                    nc.any.tensor_copy(
                        kT_pad[zoff:zoff + D, h, tch * 512:(tch + 1) * 512],
                        zc[:],
                    )

        # ---- phase AB: per 512-col chunk: transpose x, project to qkvT ----

        with (
            tc.tile_pool(name="xld", bufs=2) as pool_x,
            tc.tile_pool(name="xtc", bufs=2) as pool_xt,
            tc.tile_pool(name="ptr", bufs=3, space="PSUM") as pool_tr,
            tc.tile_pool(name="wtmp", bufs=2) as wtmp,
        ):
            for tch in range(TCH):
                xT_chunk = pool_xt.tile([P, CB, 512], F32R)
                for tbl in range(4):
                    tb = tch * 4 + tbl
                    x_tb = pool_x.tile([P, C], F32)
                    nc.sync.dma_start(x_tb[:], x[tb * P:(tb + 1) * P, :])
                    for cbp in range(2):   # pack 4 transposes per psum bank
                        pst = pool_tr.tile([P, 512], F32, tag="tr")
                        for k in range(4):
                            cb = cbp * 4 + k
                            nc.tensor.transpose(
                                pst[:, k * P:(k + 1) * P],
                                x_tb[:, cb * P:(cb + 1) * P],
                                ident[:],
                            )
                        for k in range(4):
                            cb = cbp * 4 + k
                            nc.any.tensor_copy(
                                xT_chunk[:, cb, tbl * P:(tbl + 1) * P],
                                pst[:, k * P:(k + 1) * P],
                            )
                if tch == 0:
                    # weight loads queue behind the first x tiles on DMA
                    for cb in range(CB):
                        t_ = wtmp.tile([P, NQKV], F32, tag="wld")
                        nc.sync.dma_start(t_[:], w3[cb * P:(cb + 1) * P, :])
                        nc.any.tensor_copy(w_r[:, cb, :], t_[:])
                for mb in range(MB):
                    psq = pool_mm.tile([P, 512], F32, tag="mm")
                    for cb in range(CB):
                        nc.tensor.matmul(
                            psq[:],
                            w_r[:, cb, mb * P:(mb + 1) * P],
                            xT_chunk[:, cb, :],
                            start=(cb == 0),
                            stop=(cb == CB - 1),
                        )
                    ts512 = slice(tch * 512, (tch + 1) * 512)
                    if mb < 2:
                        nc.scalar.activation(
                            qT_pair[:, mb, ts512], psq[:],
                            AF.Identity, bias=b_sb[:, mb:mb + 1], scale=1.0,
                        )
                    elif mb < 4:
                        for half in range(2):
                            h = (mb - 2) * 2 + half
                            nc.scalar.activation(
                                kT_pad[half * D:(half + 1) * D, h, ts512],
                                psq[half * D:(half + 1) * D, :],
                                AF.Identity, bias=b_sb[half * D:(half + 1) * D,
                                                       mb:mb + 1],
                                scale=1.0,
                            )
                    else:
                        nc.scalar.activation(
                            vT_tmp[:, mb - 4, ts512], psq[:],
                            AF.Identity, bias=b_sb[:, mb:mb + 1], scale=1.0,
                        )
                # v rows of this 512-col chunk -> natural layout
                # (head pairs interleaved so base-0/base-64 transposes overlap)
                for hp2 in range(2):
                    psv = {}
                    for h in (2 * hp2, 2 * hp2 + 1):
                        psv[h] = pool_tr.tile(
                            [P, 512], F32R, tag="tr", name=f"psv{h}"
                        )
                    for k in range(4):
                        jb = tch * 4 + k
                        for h in (2 * hp2, 2 * hp2 + 1):
                            base = D * (h % 2)
                            vT_h = vT_tmp[base:base + D, h // 2, :]
                            nc.tensor.transpose(
                                psv[h][:, k * D:(k + 1) * D],
                                vT_h[:, jb * P:(jb + 1) * P],
                                id2_r[base:base + D, :],
                                tile_position=(base, 0),
                            )
                    for k in range(4):
                        jb = tch * 4 + k
                        for h in (2 * hp2, 2 * hp2 + 1):
                            off = (jb * HL + h) * (D + 1)
                            nc.any.tensor_copy(
                                v1[:, off:off + D],
                                psv[h][:, k * D:(k + 1) * D],
                            )
                for k in range(4):
                    jb = tch * 4 + k
                    nc.any.tensor_copy(
                        v1[:, jb * HL * (D + 1) + D:
                              (jb * HL + HL) * (D + 1):D + 1].rearrange(
                            "p h -> p h"),
                        ones4_r[:],
                    )

        ab_ctx.close()

        wo_pair = persist.tile([P, 2, C], F32R)
        with tc.tile_pool(name="wotmp", bufs=2) as wtmp2:
            for hp in range(2):
                t_ = wtmp2.tile([P, C], F32, tag="wold")
                nc.sync.dma_start(t_[:], wo[hp * P:(hp + 1) * P, :])
                nc.any.tensor_copy(wo_pair[:, hp, :], t_[:])

        # ---- phase C: attention, software-pipelined waves ----
        # wave = (head-pair, 512-col i-chunk). While wave W's scores stream
        # through PE->ACT(exp), wave W-1's attn_v matmuls keep PE dense.
        # Raw exp tiles go to DRAM in groups of 4 j-blocks (one 3D DMA);
        # the host folds the softmax division into its transpose pass.
        # attn_v is normalized on device, so o is exact. The o-projection
        # for chunk c is emitted one wave after both head-pairs' avT for c
        # are ready, keeping PE dense to the end.
        # avT[0:64, hp, :] = even head of pair hp; [64:128, hp, :] = odd head
        avT = persist.tile([P, 2, T], F32R)
        waves = [(hp, c) for hp in range(2) for c in range(4)]

        def emit_av(Wm1, jb, psav, eTus):
            hp, c = Wm1
            njb = 4 * c + 4
            for h in (2 * hp, 2 * hp + 1):
                off = (jb * HL + h) * (D + 1)
                nc.tensor.matmul(
                    psav[h][:],
                    v1[:, off:off + P],
                    eTus[h][jb // 4][:, jb % 4, :],
                    start=(jb == 0), stop=(jb == njb - 1),
                )

        def emit_tail(Wm1, psav):
            hp, c = Wm1
            for h in (2 * hp, 2 * hp + 1):
                sums_sb = srow.tile([D + 1, 512], F32, tag="sums",
                                    name=f"sums_{h}_{c}")
                nc.vector.tensor_copy(sums_sb[D:D + 1, :], psav[h][D:D + 1, :])
                rsp = srow.tile([P, 4], F32, tag="rsp", name=f"rsp_{h}_{c}")
                nc.sync.dma_start(rsp[:], sums_sb[D:D + 1, :])
                nc.vector.reciprocal(rsp[:], rsp[:])
                row0 = srow.tile([1, 512], F32, tag="row0", name=f"row0_{h}_{c}")
                nc.sync.dma_start(row0[:], rsp[:])
                rb = srow.tile([P, 512], F32, tag="rb", name=f"rb_{h}_{c}")
                nc.gpsimd.partition_broadcast(rb[:], row0[:])
                k = h % 2
                nc.vector.tensor_tensor(
                    avT[k * D:(k + 1) * D, hp, c * 512:(c + 1) * 512],
                    psav[h][0:D, :], rb[0:D, :], ALU.mult,
                )

        def emit_o(c, pool_o):
            for tb in range(4 * c, 4 * c + 4):
                for ncol in range(2):
                    pso = pool_mm.tile([P, 512], F32, tag="mm",
                                       name=f"pso_{tb}_{ncol}")
                    for hp in range(2):
                        nc.tensor.matmul(
                            pso[:],
                            avT[:, hp, tb * P:(tb + 1) * P],
                            wo_pair[:, hp, ncol * 512:(ncol + 1) * 512],
                            start=(hp == 0), stop=(hp == 1),
                        )
                    o_sb = pool_o.tile([P, 512], F32, tag="osb",
                                       name=f"osb_{tb}_{ncol}")
                    nc.vector.tensor_copy(o_sb[:], pso[:])
                    nc.sync.dma_start(
                        o_part[tb * P:(tb + 1) * P, ncol * 512:(ncol + 1) * 512],
                        o_sb[:],
                    )

        with tc.tile_pool(name="eTu", bufs=11) as pool_eT, \
             tc.tile_pool(name="osbp", bufs=3) as pool_o, \
             tc.tile_pool(name="pav", bufs=4, space="PSUM") as pool_av:
            prev = None   # (W, psav, eTus)
            pending_o = None
            for W in waves:
                hp, c = W
                heads = (2 * hp, 2 * hp + 1)
                rhs_q = qT_pair[:, hp, :]
                njb = 4 * c + 4
                njb_prev = (4 * prev[0][1] + 4) if prev else 0
                if pending_o is not None:
                    emit_o(pending_o, pool_o)
                    pending_o = None
                psav = {h: pool_av.tile([P, 512], F32, tag="av",
                                        name=f"psav_{h}_{c}")
                        for h in heads}
                eTus = {h: [] for h in heads}
                for jb in range(max(njb, njb_prev)):
                    if jb < njb:
                        g = jb // 4
                        if jb % 4 == 0:
                            for h in heads:
                                eTus[h].append(pool_eT.tile(
                                    [P, 4, 512], F32R, tag="eTu",
                                    name=f"eTu_{h}_{c}_{g}"))
                        pss = {}
                        for h in heads:
                            ps = pool_mm.tile([P, 512], F32, tag="mm",
                                              name=f"pss_{h}_{c}_{jb}")
                            nc.tensor.matmul(
                                ps[:],
                                kT_pad[:, h, jb * P:(jb + 1) * P],
                                rhs_q[:, c * 512:(c + 1) * 512],
                                start=True, stop=True,
                            )
                            pss[h] = ps
                        p = jb - 4 * c
                        for h in heads:
                            ps = pss[h]
                            if p >= 0:
                                if p > 0:
                                    nc.vector.tensor_scalar_add(
                                        ps[:, :p * P], ps[:, :p * P], NEG
                                    )
                                nc.vector.tensor_tensor(
                                    ps[:, p * P:(p + 1) * P],
                                    ps[:, p * P:(p + 1) * P],
                                    maskT[:], ALU.add,
                                )
                            nc.scalar.activation(
                                eTus[h][g][:, jb % 4, :], ps[:],
                                AF.Exp, scale=0.125
                            )
                        if jb % 4 == 3:
                            for h in heads:
                                nc.sync.dma_start(
                                    attn4T[h, g * 512:(g + 1) * 512,
                                           c * 512:(c + 1) * 512].rearrange(
                                        "(jb p) n -> p jb n", p=P),
                                    eTus[h][g][:].bitcast(F32),
                                )
                    if prev is not None and jb < njb_prev:
                        emit_av(prev[0], jb, prev[1], prev[2])
                if prev is not None:
                    emit_tail(prev[0], prev[1])
                    if prev[0][0] == 1:
                        pending_o = prev[0][1]
                prev = (W, psav, eTus)
            # drain last wave
            if pending_o is not None:
                emit_o(pending_o, pool_o)
            for jb in range(4 * prev[0][1] + 4):
                emit_av(prev[0], jb, prev[1], prev[2])
            emit_tail(prev[0], prev[1])
            emit_o(prev[0][1], pool_o)

    nc.compile()
    return nc


_NC_CACHE = []


def _get_nc():
    if not _NC_CACHE:
        _NC_CACHE.append(build_nc())
    return _NC_CACHE[0]


def _host_consts():
    ident = np.eye(P, dtype=np.float32)
    id2 = np.concatenate([np.eye(D, dtype=np.float32)] * 2, axis=0)
    maskT = np.tril(np.full((P, P), NEG, dtype=np.float32), -1)
    ones4 = np.ones((P, HL), dtype=np.float32)
    return ident, id2, maskT, ones4


def kernel(x, w_qkv, b_qkv, w_o, b_o, _trace=False, _trace_kwargs=None):
    x = np.ascontiguousarray(np.asarray(x, dtype=np.float32))
    w_qkv = np.asarray(w_qkv, dtype=np.float32)
    b_qkv = np.asarray(b_qkv, dtype=np.float32)
    w_o = np.asarray(w_o, dtype=np.float32)
    b_o = np.asarray(b_o, dtype=np.float32)

    H = 16
    ident, id2, maskT, ones4 = _host_consts()
    in_maps = []
    for core in range(8):
        b = core // 4
        hg = (core % 4) * HL
        cols = np.r_[hg * D:(hg + HL) * D]
        w3 = np.concatenate(
            [w_qkv[:, cols], w_qkv[:, C + cols], w_qkv[:, 2 * C + cols]], axis=1
        )
        b3 = np.concatenate(
            [b_qkv[cols], b_qkv[C + cols], b_qkv[2 * C + cols]]
        )
        in_maps.append({
            "x": np.ascontiguousarray(x[b]),
            "w3": np.ascontiguousarray(w3),
            "b3": np.ascontiguousarray(b3),
            "wo": np.ascontiguousarray(w_o[hg * D:(hg + HL) * D, :]),
            "ident": ident,
            "id2": id2,
            "maskT": maskT,
            "ones4": ones4,
        })

    nc = _get_nc()
    kw = {}
    if _trace:
        kw = dict(trace=True, **(_trace_kwargs or {}))
    res = bass_utils.run_bass_kernel_spmd(
        nc, in_maps, core_ids=list(range(8)), **kw
    )

    attn_w = np.empty((2, H, T, T), dtype=np.float32)
    o = np.zeros((2, T, C), dtype=np.float32)
    for core in range(8):
        b = core // 4
        hg = (core % 4) * HL
        r = res.results[core]
        for h in range(HL):
            aw = np.ascontiguousarray(r["attn4T"][h].T)   # [i, j] raw exp
            s = aw.sum(axis=1, keepdims=True)
            np.divide(aw, s, out=aw)
            attn_w[b, hg + h] = aw
        o[b] += r["o_part"]
    o += b_o

    if _trace:
        return (o, attn_w), res
    return o, attn_w


# revision 33
# speedup vs baseline: 1.0987x; 1.0266x over previous
"""Causal self-attention on 8 TRN2 NeuronCores.

Sharding: batch (2) x head-groups (4 heads each) -> 8 cores. Each core
computes the qkv projection for its 4 heads, causal attention over
lower-triangular 128-blocks, and a partial o-projection. Host transposes the
per-head attention maps (device emits them j-major), sums o partials across
head groups, and adds the output bias.

Scores are computed only in transposed layout [j, i]: softmax row-sums come
from a ones-column appended to V (matmul row 64), 1/sum is broadcast across
partitions with gpsimd, and normalization is a tensor-tensor multiply. This
keeps the PE stream pure fp32r matmuls (no transpose-mode ops that confuse
the HAM clock gate) and each attention element passes through ACT exactly
once (the exp) plus one DVE multiply.

Self-contained: hardcodes B=2, T=2048, C=1024, H=16, D=64.
"""

import numpy as np
from contextlib import ExitStack

import concourse.bass as bass
import concourse.tile as tile
import concourse.mybir as mybir
from concourse import bacc
import concourse.bass_utils as bass_utils

F32 = mybir.dt.float32
F32R = mybir.dt.float32r
AF = mybir.ActivationFunctionType
ALU = mybir.AluOpType

P = 128
T = 2048
C = 1024
D = 64
HL = 4          # local heads per core
NQKV = 3 * HL * D   # 768
TB = T // P     # 16 row blocks
CB = C // P     # 8 contraction chunks for qkv
MB = NQKV // P  # 6 qkv output row-blocks
TCH = T // 512  # 4 column chunks of 512
NEG = -1e30


def build_nc():
    nc = bacc.Bacc("TRN2", target_bir_lowering=False, debug=False)

    x = nc.dram_tensor("x", [T, C], F32, kind="ExternalInput").ap()
    w3 = nc.dram_tensor("w3", [C, NQKV], F32, kind="ExternalInput").ap()
    b3 = nc.dram_tensor("b3", [NQKV], F32, kind="ExternalInput").ap()
    wo = nc.dram_tensor("wo", [HL * D, C], F32, kind="ExternalInput").ap()
    ident_in = nc.dram_tensor("ident", [P, P], F32, kind="ExternalInput").ap()
    id2_in = nc.dram_tensor("id2", [P, D], F32, kind="ExternalInput").ap()
    maskT_in = nc.dram_tensor("maskT", [P, P], F32, kind="ExternalInput").ap()
    ones4_in = nc.dram_tensor("ones4", [P, HL], F32, kind="ExternalInput").ap()

    # attention, transposed per head: attn4T[h, j, i] = attn_w[h, i, j]
    attn4T = nc.dram_tensor("attn4T", [HL, T, T], F32, kind="ExternalOutput").ap()
    o_part = nc.dram_tensor("o_part", [T, C], F32, kind="ExternalOutput").ap()

    with ExitStack() as ctx:
        tc = ctx.enter_context(tile.TileContext(nc))
        const = ctx.enter_context(tc.tile_pool(name="const", bufs=1))
        persist = ctx.enter_context(tc.tile_pool(name="persist", bufs=1))
        srow = ctx.enter_context(tc.tile_pool(name="srow", bufs=2))
        pool_mm = ctx.enter_context(tc.tile_pool(name="pmm", bufs=4, space="PSUM"))

        # ---- constants ----
        ident = const.tile([P, P], F32)
        nc.sync.dma_start(ident[:], ident_in)
        id2 = const.tile([P, D], F32)
        nc.sync.dma_start(id2[:], id2_in)
        id2_r = const.tile([P, D], F32R)
        nc.vector.tensor_copy(id2_r[:], id2[:])
        ident_r = const.tile([P, P], F32R)
        nc.vector.tensor_copy(ident_r[:], ident[:])
        maskT = const.tile([P, P], F32)
        nc.sync.dma_start(maskT[:], maskT_in)
        ones4 = const.tile([P, HL], F32)
        nc.sync.dma_start(ones4[:], ones4_in)
        ones4_r = const.tile([P, HL], F32R)
        nc.vector.tensor_copy(ones4_r[:], ones4[:])

        b_sb = const.tile([P, MB], F32)
        nc.sync.dma_start(b_sb[:], b3.rearrange("(o p) -> p o", p=P))

        # rounded weights (fp32r matmul operands must be compute-produced)
        qT_pair = persist.tile([P, 2, T], F32R)   # [q0q1 | q2q3]
        # per-head kT padded to K=128 with zeros in the other head's rows, so
        # score matmuls light up the full PE array (keeps the HAM clock warm)
        kT_pad = persist.tile([P, HL, T], F32R)
        # v natural + ones col; padded so lhsT can over-read to M=128 (junk
        # output rows 65..127 of psav are never consumed)
        v1 = persist.tile([P, TB * HL * (D + 1) + P, ], F32R)

        ab_ctx = ExitStack()
        pool_w = ab_ctx.enter_context(tc.tile_pool(name="pw", bufs=1))
        pool_vt = ab_ctx.enter_context(tc.tile_pool(name="pvt", bufs=1))
        w_r = pool_w.tile([P, CB, NQKV], F32R)
        vT_tmp = pool_vt.tile([P, 2, T], F32R)

        with tc.tile_pool(name="zp", bufs=1) as zpool:
            zc = zpool.tile([D, 512], F32)
            nc.gpsimd.memset(zc[:], 0.0)
            for h in range(HL):
                zoff = 0 if h % 2 else D
                for tch in range(TCH):
                    nc.any.tensor_copy(
                        kT_pad[zoff:zoff + D, h, tch * 512:(tch + 1) * 512],
                        zc[:],
                    )

        # ---- phase AB: per 512-col chunk: transpose x, project to qkvT ----

        with (
            tc.tile_pool(name="xld", bufs=2) as pool_x,
            tc.tile_pool(name="xtc", bufs=2) as pool_xt,
            tc.tile_pool(name="ptr", bufs=3, space="PSUM") as pool_tr,
            tc.tile_pool(name="wtmp", bufs=2) as wtmp,
        ):
            for tch in range(TCH):
                xT_chunk = pool_xt.tile([P, CB, 512], F32R)
                for tbl in range(4):
                    tb = tch * 4 + tbl
                    x_tb = pool_x.tile([P, C], F32)
                    nc.sync.dma_start(x_tb[:], x[tb * P:(tb + 1) * P, :])
                    for cbp in range(2):   # pack 4 transposes per psum bank
                        pst = pool_tr.tile([P, 512], F32, tag="tr")
                        for k in range(4):
                            cb = cbp * 4 + k
                            nc.tensor.transpose(
                                pst[:, k * P:(k + 1) * P],
                                x_tb[:, cb * P:(cb + 1) * P],
                                ident[:],
                            )
                        for k in range(4):
                            cb = cbp * 4 + k
                            nc.any.tensor_copy(
                                xT_chunk[:, cb, tbl * P:(tbl + 1) * P],
                                pst[:, k * P:(k + 1) * P],
                            )
                if tch == 0:
                    # weight loads queue behind the first x tiles on DMA
                    for cb in range(CB):
                        t_ = wtmp.tile([P, NQKV], F32, tag="wld")
                        nc.sync.dma_start(t_[:], w3[cb * P:(cb + 1) * P, :])
                        nc.any.tensor_copy(w_r[:, cb, :], t_[:])
                for mb in range(MB):
                    psq = pool_mm.tile([P, 512], F32, tag="mm")
                    for cb in range(CB):
                        nc.tensor.matmul(
                            psq[:],
                            w_r[:, cb, mb * P:(mb + 1) * P],
                            xT_chunk[:, cb, :],
                            start=(cb == 0),
                            stop=(cb == CB - 1),
                        )
                    ts512 = slice(tch * 512, (tch + 1) * 512)
                    if mb < 2:
                        nc.scalar.activation(
                            qT_pair[:, mb, ts512], psq[:],
                            AF.Identity, bias=b_sb[:, mb:mb + 1], scale=1.0,
                        )
                    elif mb < 4:
                        for half in range(2):
                            h = (mb - 2) * 2 + half
                            nc.scalar.activation(
                                kT_pad[half * D:(half + 1) * D, h, ts512],
                                psq[half * D:(half + 1) * D, :],
                                AF.Identity, bias=b_sb[half * D:(half + 1) * D,
                                                       mb:mb + 1],
                                scale=1.0,
                            )
                    else:
                        nc.scalar.activation(
                            vT_tmp[:, mb - 4, ts512], psq[:],
                            AF.Identity, bias=b_sb[:, mb:mb + 1], scale=1.0,
                        )
                # v rows of this 512-col chunk -> natural layout
                # (head pairs interleaved so base-0/base-64 transposes overlap)
                for hp2 in range(2):
                    psv = {}
                    for h in (2 * hp2, 2 * hp2 + 1):
                        psv[h] = pool_tr.tile(
                            [P, 512], F32R, tag="tr", name=f"psv{h}"
                        )
                    for k in range(4):
                        jb = tch * 4 + k
                        for h in (2 * hp2, 2 * hp2 + 1):
                            base = D * (h % 2)
                            vT_h = vT_tmp[base:base + D, h // 2, :]
                            nc.tensor.transpose(
                                psv[h][:, k * D:(k + 1) * D],
                                vT_h[:, jb * P:(jb + 1) * P],
                                id2_r[base:base + D, :],
                                tile_position=(base, 0),
                            )
                    for k in range(4):
                        jb = tch * 4 + k
                        for h in (2 * hp2, 2 * hp2 + 1):
                            off = (jb * HL + h) * (D + 1)
                            nc.any.tensor_copy(
                                v1[:, off:off + D],
                                psv[h][:, k * D:(k + 1) * D],
                            )
                for k in range(4):
                    jb = tch * 4 + k
                    nc.any.tensor_copy(
                        v1[:, jb * HL * (D + 1) + D:
                              (jb * HL + HL) * (D + 1):D + 1].rearrange(
                            "p h -> p h"),
                        ones4_r[:],
                    )

        ab_ctx.close()

        wo_pair = persist.tile([P, 2, C], F32R)
        with tc.tile_pool(name="wotmp", bufs=2) as wtmp2:
            for hp in range(2):
                t_ = wtmp2.tile([P, C], F32, tag="wold")
                nc.sync.dma_start(t_[:], wo[hp * P:(hp + 1) * P, :])
                nc.any.tensor_copy(wo_pair[:, hp, :], t_[:])

        # ---- phase C: attention, software-pipelined waves ----
        # wave = (head-pair, 512-col i-chunk). While wave W's scores stream
        # through PE->ACT(exp), wave W-1's attn_v matmuls keep PE dense.
        # Raw exp tiles go to DRAM in groups of 4 j-blocks (one 3D DMA);
        # the host folds the softmax division into its transpose pass.
        # attn_v is normalized on device, so o is exact. The o-projection
        # for chunk c is emitted one wave after both head-pairs' avT for c
        # are ready, keeping PE dense to the end.
        # avT[0:64, hp, :] = even head of pair hp; [64:128, hp, :] = odd head
        avT = persist.tile([P, 2, T], F32R)
        waves = [(hp, c) for hp in range(2) for c in range(4)]

        def emit_av(Wm1, jb, psav, eTus):
            hp, c = Wm1
            njb = 4 * c + 4
            for h in (2 * hp, 2 * hp + 1):
                off = (jb * HL + h) * (D + 1)
                nc.tensor.matmul(
                    psav[h][:],
                    v1[:, off:off + P],
                    eTus[h][jb // 4][:, jb % 4, :],
                    start=(jb == 0), stop=(jb == njb - 1),
                )

        def emit_tail(Wm1, psav):
            hp, c = Wm1
            for h in (2 * hp, 2 * hp + 1):
                sums_sb = srow.tile([D + 1, 512], F32, tag="sums",
                                    name=f"sums_{h}_{c}")
                nc.vector.tensor_copy(sums_sb[D:D + 1, :], psav[h][D:D + 1, :])
                rsp = srow.tile([P, 4], F32, tag="rsp", name=f"rsp_{h}_{c}")
                nc.sync.dma_start(rsp[:], sums_sb[D:D + 1, :])
                nc.vector.reciprocal(rsp[:], rsp[:])
                row0 = srow.tile([1, 512], F32, tag="row0", name=f"row0_{h}_{c}")
                nc.sync.dma_start(row0[:], rsp[:])
                rb = srow.tile([P, 512], F32, tag="rb", name=f"rb_{h}_{c}")
                nc.gpsimd.partition_broadcast(rb[:], row0[:])
                k = h % 2
                nc.vector.tensor_tensor(
                    avT[k * D:(k + 1) * D, hp, c * 512:(c + 1) * 512],
                    psav[h][0:D, :], rb[0:D, :], ALU.mult,
                )

        def emit_o(c, pool_o):
            for tb in range(4 * c, 4 * c + 4):
                for ncol in range(2):
                    pso = pool_mm.tile([P, 512], F32, tag="mm",
                                       name=f"pso_{tb}_{ncol}")
                    for hp in range(2):
                        nc.tensor.matmul(
                            pso[:],
                            avT[:, hp, tb * P:(tb + 1) * P],
                            wo_pair[:, hp, ncol * 512:(ncol + 1) * 512],
                            start=(hp == 0), stop=(hp == 1),
                        )
                    o_sb = pool_o.tile([P, 512], F32, tag="osb",
                                       name=f"osb_{tb}_{ncol}")
                    nc.vector.tensor_copy(o_sb[:], pso[:])
                    nc.sync.dma_start(
                        o_part[tb * P:(tb + 1) * P, ncol * 512:(ncol + 1) * 512],
                        o_sb[:],
                    )

        with tc.tile_pool(name="eTu", bufs=11) as pool_eT, \
             tc.tile_pool(name="osbp", bufs=3) as pool_o, \
             tc.tile_pool(name="pav", bufs=4, space="PSUM") as pool_av:
            prev = None   # (W, psav, eTus)
            pending_o = None
            for W in waves:
                hp, c = W
                heads = (2 * hp, 2 * hp + 1)
                rhs_q = qT_pair[:, hp, :]
                njb = 4 * c + 4
                njb_prev = (4 * prev[0][1] + 4) if prev else 0
                if pending_o is not None:
                    emit_o(pending_o, pool_o)
                    pending_o = None
                psav = {h: pool_av.tile([P, 512], F32, tag="av",
                                        name=f"psav_{h}_{c}")
                        for h in heads}
                eTus = {h: [] for h in heads}
                for jb in range(max(njb, njb_prev)):
                    if jb < njb:
                        g = jb // 4
                        if jb % 4 == 0:
                            for h in heads:
                                eTus[h].append(pool_eT.tile(
                                    [P, 4, 512], F32R, tag="eTu",
                                    name=f"eTu_{h}_{c}_{g}"))
                        pss = {}
                        for h in heads:
                            ps = pool_mm.tile([P, 512], F32, tag="mm",
                                              name=f"pss_{h}_{c}_{jb}")
                            nc.tensor.matmul(
                                ps[:],
                                kT_pad[:, h, jb * P:(jb + 1) * P],
                                rhs_q[:, c * 512:(c + 1) * 512],
                                start=True, stop=True,
                            )
                            pss[h] = ps
                        p = jb - 4 * c
                        for h in heads:
                            ps = pss[h]
                            if p >= 0:
                                if p > 0:
                                    nc.vector.tensor_scalar_add(
                                        ps[:, :p * P], ps[:, :p * P], NEG
                                    )
                                nc.vector.tensor_tensor(
                                    ps[:, p * P:(p + 1) * P],
                                    ps[:, p * P:(p + 1) * P],
                                    maskT[:], ALU.add,
                                )
                            nc.scalar.activation(
                                eTus[h][g][:, jb % 4, :], ps[:],
                                AF.Exp, scale=0.125
                            )
                        if jb % 4 == 3:
                            for h in heads:
                                nc.sync.dma_start(
                                    attn4T[h, g * 512:(g + 1) * 512,
                                           c * 512:(c + 1) * 512].rearrange(
                                        "(jb p) n -> p jb n", p=P),
                                    eTus[h][g][:].bitcast(F32),
                                )
                    if prev is not None and jb < njb_prev:
                        emit_av(prev[0], jb, prev[1], prev[2])
                if prev is not None:
                    emit_tail(prev[0], prev[1])
                    if prev[0][0] == 1:
                        pending_o = prev[0][1]
                prev = (W, psav, eTus)
            # drain last wave
            if pending_o is not None:
                emit_o(pending_o, pool_o)
            for jb in range(4 * prev[0][1] + 4):
                emit_av(prev[0], jb, prev[1], prev[2])
            emit_tail(prev[0], prev[1])
            emit_o(prev[0][1], pool_o)

    nc.compile()
    return nc


_NC_CACHE = []


def _get_nc():
    if not _NC_CACHE:
        _NC_CACHE.append(build_nc())
    return _NC_CACHE[0]


def _host_consts():
    ident = np.eye(P, dtype=np.float32)
    id2 = np.concatenate([np.eye(D, dtype=np.float32)] * 2, axis=0)
    maskT = np.tril(np.full((P, P), NEG, dtype=np.float32), -1)
    ones4 = np.ones((P, HL), dtype=np.float32)
    return ident, id2, maskT, ones4


def kernel(x, w_qkv, b_qkv, w_o, b_o, _trace=False, _trace_kwargs=None):
    x = np.ascontiguousarray(np.asarray(x, dtype=np.float32))
    w_qkv = np.asarray(w_qkv, dtype=np.float32)
    b_qkv = np.asarray(b_qkv, dtype=np.float32)
    w_o = np.asarray(w_o, dtype=np.float32)
    b_o = np.asarray(b_o, dtype=np.float32)

    H = 16
    ident, id2, maskT, ones4 = _host_consts()
    in_maps = []
    for core in range(8):
        b = core // 4
        hg = (core % 4) * HL
        cols = np.r_[hg * D:(hg + HL) * D]
        w3 = np.concatenate(
            [w_qkv[:, cols], w_qkv[:, C + cols], w_qkv[:, 2 * C + cols]], axis=1
        )
        b3 = np.concatenate(
            [b_qkv[cols], b_qkv[C + cols], b_qkv[2 * C + cols]]
        )
        in_maps.append({
            "x": np.ascontiguousarray(x[b]),
            "w3": np.ascontiguousarray(w3),
            "b3": np.ascontiguousarray(b3),
            "wo": np.ascontiguousarray(w_o[hg * D:(hg + HL) * D, :]),
            "ident": ident,
            "id2": id2,
            "maskT": maskT,
            "ones4": ones4,
        })

    nc = _get_nc()
    kw = {}
    if _trace:
        kw = dict(trace=True, **(_trace_kwargs or {}))
    res = bass_utils.run_bass_kernel_spmd(
        nc, in_maps, core_ids=list(range(8)), **kw
    )

    attn_w = np.empty((2, H, T, T), dtype=np.float32)
    o = np.zeros((2, T, C), dtype=np.float32)
    for core in range(8):
        b = core // 4
        hg = (core % 4) * HL
        r = res.results[core]
        for h in range(HL):
            aw = np.ascontiguousarray(r["attn4T"][h].T)   # [i, j] raw exp
            s = aw.sum(axis=1, keepdims=True)
            np.divide(aw, s, out=aw)
            attn_w[b, hg + h] = aw
        o[b] += r["o_part"]
    o += b_o

    if _trace:
        return (o, attn_w), res
    return o, attn_w
